# revision 1
# baseline (speedup 1.0000x reference)
"""Cross-attention Trainium2 kernel (8 NeuronCores, SPMD).

Reference computation (all f32):
    q = x @ Wq + bq            # [N, D]
    k = context @ Wk + bk      # [M, D]
    v = context @ Wv + bv      # [M, D]
    out = softmax(q @ k.T / sqrt(D)) @ v   # [N, D]

Sharding: rows of x (the N axis) are split across the 8 cores; context /
weights are replicated.  Each core computes its [N/8, D] slice of the
output; the host concatenates.

Device algorithm per core (all matmuls bf16 with f32 PSUM accumulation):
  - host pre-transposes x and context (and casts to bf16), so the kernel
    receives xT [D, N/8] and ctxT [D, M] in the layouts the TensorEngine
    needs (contraction dim on partitions).
  - qT = Wq.T @ xT (+bq)  computed once, kept in SBUF.
  - loop over context blocks of MB=1024 rows:
      kT_B = Wk.T @ ctxT_B (+bk)        [D, MB]   (bias via ACT per-partition)
      v_B  = ctx_B @ Wv  (+bv)          [MB, D]   (bias via K=1 matmul trick)
      S^T  = kT_B.T @ qT = k @ qT       [MB, Nq]  (scores, transposed)
      P^T  = exp(S^T / sqrt(D))                    (no max-subtraction: scores
                                                    are provably small here)
      out_acc += P^T.T @ v_B            (P^T tile is directly the lhsT)
      l_acc   += P^T.T @ ones           (softmax denominator via matmul)
  - out = out_acc / l_acc
"""

import numpy as np
import ml_dtypes

import concourse.bass as bass
import concourse.mybir as mybir
import concourse.tile as tile
from concourse import bacc
from concourse.bass_utils import run_bass_kernel_spmd

BF16 = ml_dtypes.bfloat16
F32 = mybir.dt.float32
BF = mybir.dt.bfloat16

N_CORES = 8
DIM = 1024
N_FULL = 8192
M_FULL = 8192


def build_nc(n_shard, m_total, d, mb=1024):
    """Build the per-core Bass program (SPMD: same NEFF on all cores)."""
    assert d % 128 == 0 and n_shard % 512 == 0 and m_total % mb == 0
    assert mb % 512 == 0
    dc = d // 128          # contraction chunks
    nb = m_total // mb     # context blocks
    n_qs = n_shard // 512  # q supertiles
    scale = 1.0 / float(np.sqrt(d))

    nc = bacc.Bacc("TRN2", target_bir_lowering=False, debug=False,
                   num_devices=N_CORES)

    xT = nc.dram_tensor("xT", [d, n_shard], BF, kind="ExternalInput")
    ctxT = nc.dram_tensor("ctxT", [d, m_total], BF, kind="ExternalInput")
    wq = nc.dram_tensor("wq", [d, d], BF, kind="ExternalInput")
    wk = nc.dram_tensor("wk", [d, d], BF, kind="ExternalInput")
    wv = nc.dram_tensor("wv", [d, d], BF, kind="ExternalInput")
    bq = nc.dram_tensor("bq", [128, dc], F32, kind="ExternalInput")
    bk = nc.dram_tensor("bk", [128, dc], F32, kind="ExternalInput")
    bv = nc.dram_tensor("bv", [1, d], BF, kind="ExternalInput")
    out = nc.dram_tensor("out", [n_shard, d], F32, kind="ExternalOutput")

    # DRAM views with the partition-chunk structure: [(c p) f] -> [p c f]
    xT_v = xT.ap().rearrange("(c p) n -> p c n", p=128)
    ctxT_v = ctxT.ap().rearrange("(c p) m -> p c m", p=128)
    wq_v = wq.ap().rearrange("(c p) f -> p c f", p=128)
    wk_v = wk.ap().rearrange("(c p) f -> p c f", p=128)
    wv_v = wv.ap().rearrange("(c p) f -> p c f", p=128)

    with tile.TileContext(nc) as tc:
        with (
            tc.tile_pool(name="persist", bufs=1) as persist,
            tc.tile_pool(name="ps_s", bufs=3, space="PSUM") as ps_s,
            tc.tile_pool(name="ps_o", bufs=2, space="PSUM") as ps_o,
            tc.tile_pool(name="ps_l", bufs=1, space="PSUM") as ps_l,
        ):
            # --- persistent SBUF tensors -------------------------------
            wq_sb = persist.tile([128, dc, d], BF)
            wk_sb = persist.tile([128, dc, d], BF)
            wv_sb = persist.tile([128, dc, d], BF)
            bq_sb = persist.tile([128, dc], F32)
            bk_sb = persist.tile([128, dc], F32)
            bv_sb = persist.tile([1, d], BF)
            qT_sb = persist.tile([128, dc, n_shard], BF)
            out_acc = persist.tile([128, n_shard // 128, d], F32)
            l_acc = persist.tile([128, n_shard // 128], F32)
            ones_c = persist.tile([128, 1], BF)
            ones_r = persist.tile([1, 128], BF)

            nc.sync.dma_start(out=wq_sb[:], in_=wq_v)
            nc.sync.dma_start(out=wk_sb[:], in_=wk_v)
            nc.sync.dma_start(out=wv_sb[:], in_=wv_v)
            nc.sync.dma_start(out=bq_sb[:], in_=bq.ap())
            nc.sync.dma_start(out=bk_sb[:], in_=bk.ap())
            nc.sync.dma_start(out=bv_sb[:], in_=bv.ap())
            nc.vector.memset(ones_c[:], 1.0)
            nc.vector.memset(ones_r[:], 1.0)

            # --- q projection: qT = Wq.T @ xT + bq ---------------------
            with tc.tile_pool(name="prolog", bufs=1) as prolog:
                xT_sb = prolog.tile([128, dc, n_shard], BF)
                nc.sync.dma_start(out=xT_sb[:], in_=xT_v)
                for oc in range(dc):
                    for qh in range(n_shard // 512):
                        ps = ps_s.tile([128, 512], F32)
                        for ic in range(dc):
                            nc.tensor.matmul(
                                ps[:],
                                wq_sb[:, ic, oc * 128:(oc + 1) * 128],
                                xT_sb[:, ic, qh * 512:(qh + 1) * 512],
                                start=(ic == 0), stop=(ic == dc - 1),
                            )
                        nc.scalar.activation(
                            out=qT_sb[:, oc, qh * 512:(qh + 1) * 512],
                            in_=ps[:],
                            func=mybir.ActivationFunctionType.Identity,
                            bias=bq_sb[:, oc:oc + 1],
                        )

            with (
                tc.tile_pool(name="ctx", bufs=2) as ctx_pool,
                tc.tile_pool(name="kv", bufs=1) as kv_pool,
                tc.tile_pool(name="pt", bufs=2 * (mb // 128)) as pt_pool,
            ):
                for b in range(nb):
                    # ---- load ctxT block ------------------------------
                    ctx_sb = ctx_pool.tile([128, dc, mb], BF)
                    nc.sync.dma_start(
                        out=ctx_sb[:], in_=ctxT_v[:, :, b * mb:(b + 1) * mb])

                    # ---- kT_B = Wk.T @ ctxT_B + bk  [D, MB] -----------
                    kT_sb = kv_pool.tile([128, dc, mb], BF, tag="kT")
                    for oc in range(dc):
                        for mh in range(mb // 512):
                            ps = ps_s.tile([128, 512], F32)
                            for ic in range(dc):
                                nc.tensor.matmul(
                                    ps[:],
                                    wk_sb[:, ic, oc * 128:(oc + 1) * 128],
                                    ctx_sb[:, ic, mh * 512:(mh + 1) * 512],
                                    start=(ic == 0), stop=(ic == dc - 1),
                                )
                            nc.scalar.activation(
                                out=kT_sb[:, oc, mh * 512:(mh + 1) * 512],
                                in_=ps[:],
                                func=mybir.ActivationFunctionType.Identity,
                                bias=bk_sb[:, oc:oc + 1],
                            )

                    # ---- v_B = ctx_B @ Wv + bv  [MB, D] ---------------
                    v_sb = kv_pool.tile([128, mb // 128, d], BF, tag="v")
                    for mc in range(mb // 128):
                        for dh in range(d // 512):
                            ps = ps_s.tile([128, 512], F32)
                            for ic in range(dc):
                                nc.tensor.matmul(
                                    ps[:],
                                    ctx_sb[:, ic, mc * 128:(mc + 1) * 128],
                                    wv_sb[:, ic, dh * 512:(dh + 1) * 512],
                                    start=(ic == 0), stop=False,
                                )
                            # bias row via K=1 matmul: ones.T @ bv
                            nc.tensor.matmul(
                                ps[:],
                                ones_r[:1, :128],
                                bv_sb[:1, dh * 512:(dh + 1) * 512],
                                start=False, stop=True,
                            )
                            nc.scalar.copy(
                                out=v_sb[:, mc, dh * 512:(dh + 1) * 512],
                                in_=ps[:])

                    # ---- attention for this block ---------------------
                    for qs in range(n_qs):
                        pts = []
                        for ms in range(mb // 128):
                            ps = ps_s.tile([128, 512], F32)
                            for ic in range(dc):
                                nc.tensor.matmul(
                                    ps[:],
                                    kT_sb[:, ic, ms * 128:(ms + 1) * 128],
                                    qT_sb[:, ic, qs * 512:(qs + 1) * 512],
                                    start=(ic == 0), stop=(ic == dc - 1),
                                )
                            pt = pt_pool.tile([128, 512], BF, tag="pt")
                            nc.scalar.activation(
                                out=pt[:], in_=ps[:],
                                func=mybir.ActivationFunctionType.Exp,
                                scale=scale,
                            )
                            pts.append(pt)

                        for qc in range(4):
                            qi = qs * 4 + qc
                            po = ps_o.tile([128, d], F32)
                            pl = ps_l.tile([128, 1], F32)
                            last = mb // 128 - 1
                            for ms in range(mb // 128):
                                lhs = pts[ms][:, qc * 128:(qc + 1) * 128]
                                for dh in range(d // 512):
                                    nc.tensor.matmul(
                                        po[:, dh * 512:(dh + 1) * 512],
                                        lhs,
                                        v_sb[:, ms, dh * 512:(dh + 1) * 512],
                                        start=(ms == 0), stop=(ms == last),
                                    )
                                nc.tensor.matmul(
                                    pl[:], lhs, ones_c[:, :1],
                                    start=(ms == 0), stop=(ms == last),
                                )
                            if b == 0:
                                nc.vector.tensor_copy(
                                    out=out_acc[:, qi, :], in_=po[:])
                                nc.vector.tensor_copy(
                                    out=l_acc[:, qi:qi + 1], in_=pl[:])
                            else:
                                nc.vector.tensor_add(
                                    out=out_acc[:, qi, :],
                                    in0=out_acc[:, qi, :], in1=po[:])
                                nc.vector.tensor_add(
                                    out=l_acc[:, qi:qi + 1],
                                    in0=l_acc[:, qi:qi + 1], in1=pl[:])

                # ---- normalize + write out ----------------------------
                with tc.tile_pool(name="fin", bufs=4) as fin:
                    for qi in range(n_shard // 128):
                        linv = fin.tile([128, 1], F32, tag="linv")
                        nc.vector.reciprocal(linv[:], l_acc[:, qi:qi + 1])
                        o_sb = fin.tile([128, d], F32, tag="osb")
                        nc.vector.tensor_scalar_mul(
                            out=o_sb[:], in0=out_acc[:, qi, :], scalar1=linv[:])
                        nc.sync.dma_start(
                            out=out.ap()[qi * 128:(qi + 1) * 128, :],
                            in_=o_sb[:])

    nc.compile()
    return nc


_NC_CACHE = {}


def _get_nc(n_shard, m_total, d, mb=1024):
    key = (n_shard, m_total, d, mb)
    if key not in _NC_CACHE:
        _NC_CACHE[key] = build_nc(n_shard, m_total, d, mb)
    return _NC_CACHE[key]


def _prep_inputs(x, context, Wq, bq, Wk, bk, Wv, bv, n_cores=N_CORES):
    """Host-side layout prep: transpose + bf16 cast + per-core sharding."""
    x = np.asarray(x, np.float32)
    context = np.asarray(context, np.float32)
    n, d = x.shape
    dc = d // 128
    n_shard = n // n_cores

    xT = np.ascontiguousarray(x.T).astype(BF16)            # [D, N]
    ctxT = np.ascontiguousarray(context.T).astype(BF16)    # [D, M]
    wq_b = np.asarray(Wq, np.float32).astype(BF16)
    wk_b = np.asarray(Wk, np.float32).astype(BF16)
    wv_b = np.asarray(Wv, np.float32).astype(BF16)
    bq_g = np.ascontiguousarray(np.asarray(bq, np.float32).reshape(dc, 128).T)
    bk_g = np.ascontiguousarray(np.asarray(bk, np.float32).reshape(dc, 128).T)
    bv_r = np.asarray(bv, np.float32).astype(BF16).reshape(1, d)

    in_maps = []
    for c in range(n_cores):
        in_maps.append({
            "xT": np.ascontiguousarray(xT[:, c * n_shard:(c + 1) * n_shard]),
            "ctxT": ctxT,
            "wq": wq_b, "wk": wk_b, "wv": wv_b,
            "bq": bq_g, "bk": bk_g, "bv": bv_r,
        })
    return in_maps, n_shard


def run(x, context, Wq, bq, Wk, bk, Wv, bv, trace=False, mb=1024):
    """Run the SPMD kernel; returns (out_full, BassKernelResults)."""
    in_maps, n_shard = _prep_inputs(x, context, Wq, bq, Wk, bk, Wv, bv)
    m_total, d = np.asarray(context, np.float32).shape
    nc = _get_nc(n_shard, m_total, d, mb)
    res = run_bass_kernel_spmd(nc, in_maps, core_ids=list(range(N_CORES)),
                               trace=trace)
    out = np.concatenate([res.results[c]["out"] for c in range(N_CORES)],
                         axis=0)
    return np.asarray(out, np.float32), res


def kernel(x, context, Wq, bq, Wk, bk, Wv, bv):
    out, _ = run(x, context, Wq, bq, Wk, bk, Wv, bv, trace=False)
    return out


# revision 2
# speedup vs baseline: 1.2992x; 1.2992x over previous
"""Cross-attention Trainium2 kernel (8 NeuronCores, SPMD).

Reference computation (all f32):
    q = x @ Wq + bq            # [N, D]
    k = context @ Wk + bk      # [M, D]
    v = context @ Wv + bv      # [M, D]
    out = softmax(q @ k.T / sqrt(D)) @ v   # [N, D]

Sharding: rows of x (N axis) AND rows of context (M axis) are both split
across the 8 cores.  Each core projects its own context shard to k/v,
the shards are all-gathered in-NEFF (bf16, 2x AllGather), and each core
then computes attention for its x shard against the full gathered K/V.

Device algorithm per core (all matmuls bf16 with f32 PSUM accumulation):
  - host pre-transposes x and context (and casts to bf16), so the kernel
    receives xT [D, N/8] and ctxT [D, M/8] with the contraction dim on
    partitions.
  - kT_c = Wk.T @ ctxT_c (+bk), v_c = ctx_c @ Wv (+bv)  -> DRAM -> AllGather
  - qT = Wq.T @ xT (+bq) computed once, kept in SBUF (overlaps gather).
  - loop over gathered context blocks b (1024 rows each):
      S^T  = kT_b.T @ qT = k @ qT       [MB, Nq]  (scores, transposed)
      P^T  = exp(S^T / sqrt(D))                    (no max-subtraction: scores
                                                    are provably small here)
      out_acc += P^T.T @ v_b            (P^T tile is directly the lhsT)
      l_acc   += P^T.T @ ones           (softmax denominator via matmul)
  - out = out_acc / l_acc
"""

import numpy as np
import ml_dtypes

import concourse.bass as bass
import concourse.mybir as mybir
import concourse.tile as tile
from concourse import bacc
from concourse.bass_utils import run_bass_kernel_spmd

BF16 = ml_dtypes.bfloat16
F32 = mybir.dt.float32
BF = mybir.dt.bfloat16

N_CORES = 8
DIM = 1024
N_FULL = 8192
M_FULL = 8192


def build_nc(n_shard, m_total, d, mb=1024):
    """Build the per-core Bass program (SPMD: same NEFF on all cores)."""
    assert d % 128 == 0 and n_shard % 512 == 0
    assert m_total % N_CORES == 0
    m_shard = m_total // N_CORES
    assert m_shard == mb, "one gathered block per core shard"
    dc = d // 128          # contraction chunks
    n_qs = n_shard // 512  # q supertiles
    scale = 1.0 / float(np.sqrt(d))

    nc = bacc.Bacc("TRN2", target_bir_lowering=False, debug=False,
                   num_devices=N_CORES)

    xT = nc.dram_tensor("xT", [d, n_shard], BF, kind="ExternalInput")
    ctxT = nc.dram_tensor("ctxT", [d, m_shard], BF, kind="ExternalInput")
    wq = nc.dram_tensor("wq", [d, d], BF, kind="ExternalInput")
    wk = nc.dram_tensor("wk", [d, d], BF, kind="ExternalInput")
    wv = nc.dram_tensor("wv", [d, d], BF, kind="ExternalInput")
    bq = nc.dram_tensor("bq", [128, dc], F32, kind="ExternalInput")
    bk = nc.dram_tensor("bk", [128, dc], F32, kind="ExternalInput")
    bv = nc.dram_tensor("bv", [1, d], BF, kind="ExternalInput")
    out = nc.dram_tensor("out", [n_shard, d], F32, kind="ExternalOutput")

    # internal DRAM for the collective
    k_loc = nc.dram_tensor("k_loc", [d, m_shard], BF)
    v_loc = nc.dram_tensor("v_loc", [m_shard, d], BF)
    k_all = nc.dram_tensor("k_all", [N_CORES, d, m_shard], BF,
                           addr_space="Shared")
    v_all = nc.dram_tensor("v_all", [N_CORES, m_shard, d], BF,
                           addr_space="Shared")

    # DRAM views with the partition-chunk structure: [(c p) f] -> [p c f]
    xT_v = xT.ap().rearrange("(c p) n -> p c n", p=128)
    ctxT_v = ctxT.ap().rearrange("(c p) m -> p c m", p=128)
    wq_v = wq.ap().rearrange("(c p) f -> p c f", p=128)
    wk_v = wk.ap().rearrange("(c p) f -> p c f", p=128)
    wv_v = wv.ap().rearrange("(c p) f -> p c f", p=128)
    k_loc_v = k_loc.ap().rearrange("(c p) m -> p c m", p=128)
    v_loc_v = v_loc.ap().rearrange("(c p) f -> p c f", p=128)
    k_all_v = k_all.ap().rearrange("b (c p) m -> b p c m", p=128)
    v_all_v = v_all.ap().rearrange("b (c p) f -> b p c f", p=128)

    groups = [list(range(N_CORES))]

    with tile.TileContext(nc) as tc:
        with (
            tc.tile_pool(name="persist", bufs=1) as persist,
            tc.tile_pool(name="ps_s", bufs=2, space="PSUM") as ps_s,
            tc.tile_pool(name="ps_o", bufs=2, space="PSUM") as ps_o,
            tc.tile_pool(name="ps_l", bufs=2, space="PSUM") as ps_l,
        ):
            qT_sb = persist.tile([128, dc, n_shard], BF)
            out_acc = persist.tile([128, n_shard // 128, d], F32)
            l_acc = persist.tile([128, n_shard // 128], F32)
            ones_c = persist.tile([128, 1], BF)
            bq_sb = persist.tile([128, dc], F32)
            nc.vector.memset(ones_c[:], 1.0)
            nc.sync.dma_start(out=bq_sb[:], in_=bq.ap())

            # ---------------- phase A: k/v projection of own shard ------
            with tc.tile_pool(name="phaseA", bufs=1) as pa:
                wk_sb = pa.tile([128, dc, d], BF)
                wv_sb = pa.tile([128, dc, d], BF)
                wq_sb = pa.tile([128, dc, d], BF)
                bk_sb = pa.tile([128, dc], F32)
                bv_sb = pa.tile([1, d], BF)
                ones_r = pa.tile([1, 128], BF)
                ctx_sb = pa.tile([128, dc, m_shard], BF)
                xT_sb = pa.tile([128, dc, n_shard], BF)
                kT_c = pa.tile([128, dc, m_shard], BF)
                v_c = pa.tile([128, m_shard // 128, d], BF)

                nc.sync.dma_start(out=wk_sb[:], in_=wk_v)
                nc.sync.dma_start(out=wv_sb[:], in_=wv_v)
                nc.sync.dma_start(out=bk_sb[:], in_=bk.ap())
                nc.sync.dma_start(out=bv_sb[:], in_=bv.ap())
                nc.sync.dma_start(out=ctx_sb[:], in_=ctxT_v)
                nc.vector.memset(ones_r[:], 1.0)

                # kT_c = Wk.T @ ctxT_c + bk
                for oc in range(dc):
                    for mh in range(m_shard // 512):
                        ps = ps_s.tile([128, 512], F32)
                        for ic in range(dc):
                            nc.tensor.matmul(
                                ps[:],
                                wk_sb[:, ic, oc * 128:(oc + 1) * 128],
                                ctx_sb[:, ic, mh * 512:(mh + 1) * 512],
                                start=(ic == 0), stop=(ic == dc - 1),
                            )
                        nc.scalar.activation(
                            out=kT_c[:, oc, mh * 512:(mh + 1) * 512],
                            in_=ps[:],
                            func=mybir.ActivationFunctionType.Identity,
                            bias=bk_sb[:, oc:oc + 1],
                        )
                # v_c = ctx_c @ Wv + bv
                for mc in range(m_shard // 128):
                    for dh in range(d // 512):
                        ps = ps_s.tile([128, 512], F32)
                        for ic in range(dc):
                            nc.tensor.matmul(
                                ps[:],
                                ctx_sb[:, ic, mc * 128:(mc + 1) * 128],
                                wv_sb[:, ic, dh * 512:(dh + 1) * 512],
                                start=(ic == 0), stop=False,
                            )
                        nc.tensor.matmul(
                            ps[:], ones_r[:1, :128],
                            bv_sb[:1, dh * 512:(dh + 1) * 512],
                            start=False, stop=True,
                        )
                        nc.scalar.copy(
                            out=v_c[:, mc, dh * 512:(dh + 1) * 512], in_=ps[:])

                # stage into DRAM, then all-gather (k first: attention
                # needs kT before v)
                nc.sync.dma_start(out=k_loc_v, in_=kT_c[:])
                nc.sync.dma_start(out=v_loc_v, in_=v_c[:])
                nc.gpsimd.collective_compute(
                    "AllGather", mybir.AluOpType.bypass,
                    replica_groups=groups,
                    ins=[k_loc.ap()], outs=[k_all.ap()],
                )
                nc.gpsimd.collective_compute(
                    "AllGather", mybir.AluOpType.bypass,
                    replica_groups=groups,
                    ins=[v_loc.ap()], outs=[v_all.ap()],
                )

                # qT = Wq.T @ xT + bq  (overlaps the gather)
                nc.sync.dma_start(out=wq_sb[:], in_=wq_v)
                nc.sync.dma_start(out=xT_sb[:], in_=xT_v)
                for oc in range(dc):
                    for qh in range(n_shard // 512):
                        ps = ps_s.tile([128, 512], F32)
                        for ic in range(dc):
                            nc.tensor.matmul(
                                ps[:],
                                wq_sb[:, ic, oc * 128:(oc + 1) * 128],
                                xT_sb[:, ic, qh * 512:(qh + 1) * 512],
                                start=(ic == 0), stop=(ic == dc - 1),
                            )
                        nc.scalar.activation(
                            out=qT_sb[:, oc, qh * 512:(qh + 1) * 512],
                            in_=ps[:],
                            func=mybir.ActivationFunctionType.Identity,
                            bias=bq_sb[:, oc:oc + 1],
                        )

            # ---------------- phase B: attention over gathered blocks ---
            with (
                tc.tile_pool(name="kv", bufs=2) as kv_pool,
                tc.tile_pool(name="pt", bufs=2 * (mb // 128)) as pt_pool,
            ):
                for b in range(N_CORES):
                    kT_sb = kv_pool.tile([128, dc, mb], BF, tag="kT")
                    nc.sync.dma_start(out=kT_sb[:], in_=k_all_v[b])
                    v_sb = kv_pool.tile([128, mb // 128, d], BF, tag="v")
                    nc.sync.dma_start(out=v_sb[:], in_=v_all_v[b])

                    for qs in range(n_qs):
                        pts = []
                        for ms in range(mb // 128):
                            ps = ps_s.tile([128, 512], F32)
                            for ic in range(dc):
                                nc.tensor.matmul(
                                    ps[:],
                                    kT_sb[:, ic, ms * 128:(ms + 1) * 128],
                                    qT_sb[:, ic, qs * 512:(qs + 1) * 512],
                                    start=(ic == 0), stop=(ic == dc - 1),
                                )
                            pt = pt_pool.tile([128, 512], BF, tag="pt")
                            nc.scalar.activation(
                                out=pt[:], in_=ps[:],
                                func=mybir.ActivationFunctionType.Exp,
                                scale=scale,
                            )
                            pts.append(pt)

                        for qc in range(4):
                            qi = qs * 4 + qc
                            po = ps_o.tile([128, d], F32)
                            pl = ps_l.tile([128, 1], F32)
                            last = mb // 128 - 1
                            for ms in range(mb // 128):
                                lhs = pts[ms][:, qc * 128:(qc + 1) * 128]
                                for dh in range(d // 512):
                                    nc.tensor.matmul(
                                        po[:, dh * 512:(dh + 1) * 512],
                                        lhs,
                                        v_sb[:, ms, dh * 512:(dh + 1) * 512],
                                        start=(ms == 0), stop=(ms == last),
                                    )
                                nc.tensor.matmul(
                                    pl[:], lhs, ones_c[:, :1],
                                    start=(ms == 0), stop=(ms == last),
                                )
                            if b == 0:
                                nc.vector.tensor_copy(
                                    out=l_acc[:, qi:qi + 1], in_=pl[:])
                                nc.vector.tensor_copy(
                                    out=out_acc[:, qi, :], in_=po[:])
                            else:
                                nc.vector.tensor_add(
                                    out=l_acc[:, qi:qi + 1],
                                    in0=l_acc[:, qi:qi + 1], in1=pl[:])
                                nc.vector.tensor_add(
                                    out=out_acc[:, qi, :],
                                    in0=out_acc[:, qi, :], in1=po[:])

                # ---- normalize + write out ----------------------------
                with tc.tile_pool(name="fin", bufs=4) as fin:
                    for qi in range(n_shard // 128):
                        linv = fin.tile([128, 1], F32, tag="linv")
                        nc.vector.reciprocal(linv[:], l_acc[:, qi:qi + 1])
                        o_sb = fin.tile([128, d], F32, tag="osb")
                        nc.vector.tensor_scalar_mul(
                            out=o_sb[:], in0=out_acc[:, qi, :], scalar1=linv[:])
                        nc.sync.dma_start(
                            out=out.ap()[qi * 128:(qi + 1) * 128, :],
                            in_=o_sb[:])

    nc.compile()
    return nc


_NC_CACHE = {}


def _get_nc(n_shard, m_total, d, mb=1024):
    key = (n_shard, m_total, d, mb)
    if key not in _NC_CACHE:
        _NC_CACHE[key] = build_nc(n_shard, m_total, d, mb)
    return _NC_CACHE[key]


def _prep_inputs(x, context, Wq, bq, Wk, bk, Wv, bv, n_cores=N_CORES):
    """Host-side layout prep: transpose + bf16 cast + per-core sharding."""
    x = np.asarray(x, np.float32)
    context = np.asarray(context, np.float32)
    n, d = x.shape
    m = context.shape[0]
    dc = d // 128
    n_shard = n // n_cores
    m_shard = m // n_cores

    xT = np.ascontiguousarray(x.T).astype(BF16)            # [D, N]
    ctxT = np.ascontiguousarray(context.T).astype(BF16)    # [D, M]
    wq_b = np.asarray(Wq, np.float32).astype(BF16)
    wk_b = np.asarray(Wk, np.float32).astype(BF16)
    wv_b = np.asarray(Wv, np.float32).astype(BF16)
    bq_g = np.ascontiguousarray(np.asarray(bq, np.float32).reshape(dc, 128).T)
    bk_g = np.ascontiguousarray(np.asarray(bk, np.float32).reshape(dc, 128).T)
    bv_r = np.asarray(bv, np.float32).astype(BF16).reshape(1, d)

    in_maps = []
    for c in range(n_cores):
        in_maps.append({
            "xT": np.ascontiguousarray(xT[:, c * n_shard:(c + 1) * n_shard]),
            "ctxT": np.ascontiguousarray(
                ctxT[:, c * m_shard:(c + 1) * m_shard]),
            "wq": wq_b, "wk": wk_b, "wv": wv_b,
            "bq": bq_g, "bk": bk_g, "bv": bv_r,
        })
    return in_maps, n_shard


def run(x, context, Wq, bq, Wk, bk, Wv, bv, trace=False, mb=1024):
    """Run the SPMD kernel; returns (out_full, BassKernelResults)."""
    in_maps, n_shard = _prep_inputs(x, context, Wq, bq, Wk, bk, Wv, bv)
    m_total, d = np.asarray(context, np.float32).shape
    nc = _get_nc(n_shard, m_total, d, mb)
    res = run_bass_kernel_spmd(nc, in_maps, core_ids=list(range(N_CORES)),
                               trace=trace)
    out = np.concatenate([res.results[c]["out"] for c in range(N_CORES)],
                         axis=0)
    return np.asarray(out, np.float32), res


def kernel(x, context, Wq, bq, Wk, bk, Wv, bv):
    out, _ = run(x, context, Wq, bq, Wk, bk, Wv, bv, trace=False)
    return out
